# revision 25
# baseline (speedup 1.0000x reference)
"""GAT (2-layer, PyG-style GATConv) on 8 Trainium2 NeuronCores.

Strategy (dst-sharded, quad-packed gather):
- Nodes sharded by dst across 8 cores (12500 each); edges partitioned by dst
  core; segment-softmax + weighted aggregation local per dst shard.
- Node table packs FOUR nodes per 256B row ([4 x 16 bf16 h | 4 x fp32 a_src |
  pad]), so a single int16-indexed SWDGE gather chunk covers all 100k nodes
  (idx = node>>2, 25000 rows). One gather per 128-dst tile fetches one 256B
  quad-row per edge slot; per-slot additive masks (0 / ln(mult) / -1e30)
  select the sub-row inside the segment softmax, which also handles duplicate
  edges exactly. exp() runs without max-subtraction (|e| <~ 30, safe in fp32).
- Slots are degree-sorted per core; the static grid K[t] is the max over
  cores, while each core's gather stops early via a runtime num_idxs register
  (trailing -1 idxs are skipped by SWDGE), so descriptor count ~= edge count
  (+0.4%).
- 3 SPMD launches: transform (x@W1 + scores) / layer-1 aggregation /
  layer-2 aggregation + classifier + log_softmax. The gather schedule is
  identical for both layers; per-tile idx/mask/a_dst ride in one combined
  int16 DMA.
"""

import numpy as np

import concourse.ap_utils as ap_utils
import concourse.bacc as bacc
import concourse.bass as bass
import concourse.mybir as mybir
from concourse.bass import round_up_to_multiple
from concourse.bass_utils import run_bass_kernel_spmd
from concourse.masks import make_identity
from concourse.tile import TileContext

P = 128
NCORES = 8
N = 100000
F_IN = 512
HID = 16
C_OUT = 32
NEG_SLOPE = 0.2
NROW = N // 4          # 25000 quad rows
ROWW = 128             # bf16 elems per table row (256B)
SH = N // NCORES       # nodes per core
T_TILES = (SH + P - 1) // P
SHP = T_TILES * P      # padded shard size (12544)
NEG_BIG = -1.0e30
MAX_IDX_PER_GATHER = 8192

FP = mybir.dt.float32
BF = mybir.dt.bfloat16
I16 = mybir.dt.int16
I32 = mybir.dt.int32


def _my_dma_gather(gp, out_ap, in_ap, idxs_ap, num_idxs, num_idxs_reg,
                   elem_size, elem_step, queue_num):
    """BassGpSimd.dma_gather (non-transpose, DRAM source) without the
    256B-elem_size restriction and with a runtime num_idxs register; the row
    stride (elem_step) must still be a multiple of 256B."""
    assert idxs_ap.dtype == I16
    assert in_ap.dtype == out_ap.dtype
    assert in_ap.space == bass.MemorySpace.DRAM
    assert idxs_ap.space == bass.MemorySpace.SBUF
    assert out_ap.space == bass.MemorySpace.SBUF
    assert ap_utils.ap_is_contiguous(out_ap.ap[1:])
    assert ap_utils.ap_is_contiguous(idxs_ap.ap[1:])
    assert in_ap.ap[-1][1] == out_ap.ap[-1][1] == elem_size
    assert out_ap.ap[0][1] * out_ap.ap[1][1] == round_up_to_multiple(num_idxs, 128)
    assert in_ap.ap[0][0] == elem_step
    stride_bytes = elem_step * mybir.dt.size(in_ap.dtype)
    assert stride_bytes % 256 == 0 and stride_bytes // 256 < 256
    _in_ap = gp.lower_ap_dma(in_ap, for_custom_bir_dma=True)
    _idxs_ap = gp.lower_ap(idxs_ap)
    _out_ap = gp.lower_ap(out_ap)
    return gp.add_instruction(
        mybir.InstDMAGatherAnt(
            name=gp.bass.get_next_instruction_name(),
            ins=[*_in_ap, _idxs_ap, gp.lower_val_access(gp.to_reg(num_idxs_reg))],
            outs=[_out_ap],
            transpose=False,
            num_idxs=num_idxs,
            elem_size=elem_size,
            stride_bytes_256=stride_bytes // 256,
            gen_mode=0,
            single_packet=False,
            queue_num=queue_num,
        )
    )


# ---------------------------------------------------------------------------
# Host-side preprocessing
# ---------------------------------------------------------------------------

def _wrap_idx(local_idx):
    """Wrap an int16 index list [M] (M % 128 == 0) into the SWDGE layout
    [128, M//16]: idx i at partition i%16, col i//16, replicated x8."""
    M = local_idx.shape[0]
    w = local_idx.reshape(M // 16, 16).T.astype(np.int16)  # [16, M//16]
    return np.tile(w, (8, 1))


def _build_schedule(edge_index):
    """Shared (both layers) gather schedule.

    Per core returns a combined int16 tensor [128, sum(kt*16+2)] holding, per
    tile: [idx kt*8 | mask-f32-bits kt*8 | a_dst-f32-bits 2] (a_dst cols are
    filled per layer later), plus counts [1, T] i32 and the node order.
    """
    src = np.asarray(edge_index[0], dtype=np.int64)
    dst = np.asarray(edge_index[1], dtype=np.int64)
    loops = np.arange(N, dtype=np.int64)
    src = np.concatenate([src, loops])
    dst = np.concatenate([dst, loops])
    core = dst // SH

    per_core = []
    for c in range(NCORES):
        m = core == c
        s_c = src[m]
        d_loc = dst[m] - c * SH
        quad = s_c >> 2
        sub = (s_c & 3).astype(np.int64)
        # slots: unique (d_loc, quad); per-(slot, sub) multiplicity
        ekey = (d_loc * NROW + quad) * 4 + sub
        uk, ucnt = np.unique(ekey, return_counts=True)
        skey = uk >> 2
        usub = (uk & 3).astype(np.int64)
        slot_ids, slot_inv = np.unique(skey, return_inverse=True)
        nslots = len(slot_ids)
        slot_d = slot_ids // NROW
        slot_q = (slot_ids % NROW).astype(np.int64)
        mask = np.full((nslots, 4), NEG_BIG, np.float32)
        mask[slot_inv, usub] = np.log(ucnt).astype(np.float32)
        deg = np.bincount(slot_d, minlength=SH)
        per_core.append((slot_d, slot_q, mask, deg))

    # shared degree-sorted tiling
    orders = [np.argsort(-pc[3], kind="stable").astype(np.int64)
              for pc in per_core]
    Kc = np.zeros((NCORES, T_TILES), np.int64)
    for c in range(NCORES):
        deg = per_core[c][3]
        ds = deg[orders[c]]
        grid = np.concatenate([ds, np.zeros(SHP - SH, np.int64)]).reshape(T_TILES, P)
        Kc[c] = grid.max(axis=1)
    K = np.maximum(Kc.max(axis=0), 1)
    assert int(K.max()) * P <= MAX_IDX_PER_GATHER
    off = np.concatenate([[0], np.cumsum(K)])       # slot-col offsets per tile
    coff = np.concatenate([[0], np.cumsum(K * 16 + 2)])  # combined col offsets

    cores_data = []
    for c in range(NCORES):
        slot_d, slot_q, mask, deg = per_core[c]
        order = orders[c]
        gridpos = np.full(SH, -1, np.int64)
        gridpos[order] = np.arange(SH)
        gp_s = gridpos[slot_d]
        t_s = gp_s // P
        p_s = gp_s % P
        so = np.argsort(gp_s * NROW + slot_q, kind="stable")
        gs = gp_s[so]
        rank = np.arange(len(gs)) - np.searchsorted(gs, gs, side="left")
        rank_s = np.empty_like(rank)
        rank_s[so] = rank

        total = int(K.sum()) * P
        idx_arr = np.full(total, -1, np.int64)
        mask_arr = np.full((total, 4), NEG_BIG, np.float32)
        pos = (off[t_s] + rank_s) * P + p_s
        idx_arr[pos] = slot_q
        mask_arr[pos] = mask

        counts = np.zeros(T_TILES, np.int32)
        comb = np.zeros((P, int(coff[-1])), np.int16)
        for t in range(T_TILES):
            kt = int(K[t])
            a = idx_arr[off[t] * P:off[t + 1] * P]
            nz = np.nonzero(a >= 0)[0]
            last = int(nz.max()) if len(nz) else 0
            a[:last + 1][a[:last + 1] < 0] = 0
            counts[t] = last + 1
            co = int(coff[t])
            comb[:, co:co + kt * 8] = _wrap_idx(a.astype(np.int16))
            mt = mask_arr[off[t] * P:off[t + 1] * P].reshape(kt, P, 4)
            mt = np.ascontiguousarray(mt.transpose(1, 0, 2)).reshape(P, kt * 4)
            comb[:, co + kt * 8:co + kt * 16] = mt.view(np.int16)
        cnt_t = np.zeros((1, T_TILES), np.int32)
        cnt_t[0] = counts
        cores_data.append({"comb": comb, "counts": cnt_t, "order": order})
    return K, coff, cores_data


def _fill_ad(cores_data, K, coff, ad_full):
    """Write per-layer a_dst values into the combined tensors; returns copies.
    ad_full: [N] fp32 in node-id order."""
    outs = []
    for c in range(NCORES):
        cd = cores_data[c]
        comb = cd["comb"].copy()
        adv = np.zeros(SHP, np.float32)
        adv[:SH] = ad_full[c * SH + cd["order"]]
        advt = adv.reshape(T_TILES, P)
        for t in range(T_TILES):
            kt = int(K[t])
            co = int(coff[t])
            comb[:, co + kt * 16:co + kt * 16 + 2] = \
                advt[t][:, None].view(np.int16)
        outs.append(comb)
    return outs


def _pack_table(h_bf16_bits, a_s):
    """h_bf16_bits [N, HID] uint16, a_s [N] float32 -> [NROW, ROWW] bf16.
    Row layout (bf16 elems): [h0|h1|h2|h3 (64) | a_s0..3 (4xf32 = 8) | pad]."""
    import ml_dtypes
    tab = np.zeros((NROW, ROWW), np.uint16)
    tab[:, 0:64] = h_bf16_bits.reshape(NROW, 4 * HID)
    tab[:, 64:72] = a_s.astype(np.float32).view(np.uint16).reshape(NROW, 8)
    return tab.view(ml_dtypes.bfloat16)


# ---------------------------------------------------------------------------
# Device programs
# ---------------------------------------------------------------------------

def _build_transform(repeat=1):
    """Launch 1: per core, h = xT_shard.T @ W1; a_s = h@att_src; a_d = h@att_dst.
    Inputs : xt [F_IN, SH] bf16 (pre-transposed shard), w1 [F_IN//P, P, HID] bf16,
             attuv [HID, 2] fp32 (att_src | att_dst)
    Outputs: hasd [SHP, HID+2] fp32 (h | a_s | a_d)
    """
    nc = bacc.Bacc("TRN2", target_bir_lowering=False, debug=False,
                   num_devices=NCORES)
    xt = nc.dram_tensor("xt", [F_IN, SH], BF, kind="ExternalInput").ap()
    w1 = nc.dram_tensor("w1", [F_IN // P, P, HID], BF, kind="ExternalInput").ap()
    attuv = nc.dram_tensor("attuv", [HID, 2], FP, kind="ExternalInput").ap()
    hasd = nc.dram_tensor("hasd", [SHP, HID + 2], FP, kind="ExternalOutput").ap()
    KC = F_IN // P
    with TileContext(nc) as tc:
        with tc.tile_pool(name="cst", bufs=1) as cst, \
             tc.tile_pool(name="xk", bufs=3) as xk, \
             tc.tile_pool(name="hp", bufs=3) as hp, \
             tc.tile_pool(name="ps", bufs=2, space="PSUM") as ps:
            w1t = cst.tile([P, KC * HID], BF)
            nc.sync.dma_start(out=w1t[:].rearrange("p (k h) -> p k h", k=KC),
                              in_=w1[:].rearrange("k p h -> p k h"))
            uvt = cst.tile([HID, 2], FP)
            nc.sync.dma_start(out=uvt[:], in_=attuv[:])
            ident = cst.tile([P, P], FP)
            make_identity(nc, ident[:])

            def tbody(t):
                m0 = t * P
                mn = min(P, SH - m0)
                xtile = xk.tile([P, KC * P], BF, tag="xt")
                nc.sync.dma_start(
                    out=xtile[:].rearrange("p (k m) -> p k m", k=KC)[:, :, 0:mn],
                    in_=xt[:, m0:m0 + mn].rearrange("(k p) m -> p k m", p=P))
                psum = ps.tile([P, HID], FP, space="PSUM", tag="ps")
                for k in range(KC):
                    nc.tensor.matmul(
                        psum[:mn, :],
                        lhsT=xtile[:, k * P:k * P + mn],
                        rhs=w1t[:, k * HID:(k + 1) * HID],
                        start=(k == 0), stop=(k == KC - 1))
                ht = hp.tile([P, HID + 2], FP, tag="ht")
                if mn < P:
                    nc.vector.memset(ht[:], 0.0)
                nc.scalar.copy(ht[:mn, 0:HID], psum[:mn, 0:HID])
                pT = ps.tile([HID, P], FP, space="PSUM", tag="pT")
                nc.tensor.transpose(pT[:], ht[:, 0:HID], ident[:])
                hT = hp.tile([HID, P], FP, tag="hT")
                nc.scalar.copy(hT[:], pT[:])
                pu = ps.tile([P, 2], FP, space="PSUM", tag="pu")
                nc.tensor.matmul(pu[:], lhsT=hT[:], rhs=uvt[:],
                                 start=True, stop=True)
                nc.scalar.copy(ht[:, HID:HID + 2], pu[:])
                nc.sync.dma_start(out=hasd[m0:m0 + P, :], in_=ht[:])

            if repeat > 1:
                with tc.For_i(0, repeat):
                    for t in range(T_TILES):
                        tbody(t)
            else:
                for t in range(T_TILES):
                    tbody(t)
    nc.compile()
    return nc


def _build_aggregate(K, coff, layer, repeat=1, bench_mode=0):
    """Launches 2 & 3: quad gather + segment softmax + weighted aggregation.

    layer == 1: out = relu(num/den + b1) -> hasd2 [SHP, HID+2] (h' | a_s2 | a_d2)
    layer == 2: out = log_softmax(num/den @ W2 + b2) -> y [SHP, C_OUT]
    Inputs: tab [NROW, ROWW] bf16; comb [128, CW] i16 (idx|mask|a_d per tile);
            cnts [1, T] i32; vecs [P, HID or C_OUT] (b1 | b2 tiled);
            uv [HID, 2] fp32 (layer 1); w2 [HID, C_OUT] fp32 (layer 2).
    repeat > 1 wraps the tile loop in a hardware loop (benchmarking).
    """
    nc = bacc.Bacc("TRN2", target_bir_lowering=False, debug=False,
                   num_devices=NCORES, num_swdge_queues=4)
    CW = int(coff[-1])
    tab = nc.dram_tensor("tab", [NROW, ROWW], BF, kind="ExternalInput").ap()
    comb = nc.dram_tensor("comb", [P, CW], I16, kind="ExternalInput").ap()
    cnts = nc.dram_tensor("cnts", [1, T_TILES], I32, kind="ExternalInput").ap()
    if layer == 1:
        vecs = nc.dram_tensor("vecs", [P, HID], FP, kind="ExternalInput").ap()
        uv = nc.dram_tensor("uv", [HID, 2], FP, kind="ExternalInput").ap()
        hasd2 = nc.dram_tensor("hasd2", [SHP, HID + 2], FP,
                               kind="ExternalOutput").ap()
    else:
        vecs = nc.dram_tensor("vecs", [P, C_OUT], FP, kind="ExternalInput").ap()
        w2 = nc.dram_tensor("w2", [HID, C_OUT], FP, kind="ExternalInput").ap()
        y = nc.dram_tensor("y", [SHP, C_OUT], FP, kind="ExternalOutput").ap()

    with TileContext(nc) as tc:
        with tc.tile_pool(name="cst", bufs=1) as cst, \
             tc.tile_pool(name="ix", bufs=5) as ixp, \
             tc.tile_pool(name="gr", bufs=5) as grp, \
             tc.tile_pool(name="sc", bufs=3) as scp, \
             tc.tile_pool(name="ou", bufs=3) as oup, \
             tc.tile_pool(name="ps", bufs=2, space="PSUM") as ps:
            vt = cst.tile([P, vecs.shape[1]], FP)
            nc.sync.dma_start(out=vt[:], in_=vecs[:])
            cntt = cst.tile([1, T_TILES], I32)
            nc.sync.dma_start(out=cntt[:], in_=cnts[:])
            ident = cst.tile([P, P], FP)
            make_identity(nc, ident[:])
            if layer == 1:
                uvt = cst.tile([HID, 2], FP)
                nc.sync.dma_start(out=uvt[:], in_=uv[:])
            else:
                w2t = cst.tile([HID, C_OUT], FP)
                nc.sync.dma_start(out=w2t[:], in_=w2[:])
            # zero-fill gather buffers once (stale SBUF could be NaN bits)
            ktmax = int(K.max())
            for b in range(5):
                gz = grp.tile([P, ktmax * ROWW], BF, tag="grid")
                nc.vector.memset(gz[:], 0.0)
            nregs = [nc.gpsimd.alloc_register(f"nidx{i}") for i in range(4)]

            GS = 7                       # tiles per log_softmax batch (L2)
            assert T_TILES % GS == 0

            def s1_gather(t):
                """Issue combined DMA + SWDGE gather for tile t."""
                kt = int(K[t])
                co = int(coff[t])
                g = grp.tile([P, kt * ROWW], BF, tag="grid", name="g")
                cmb = ixp.tile([P, kt * 16 + 2], I16, tag="cmb", name="cmb")
                nc.sync.dma_start(out=cmb[:],
                                  in_=comb[:, co:co + kt * 16 + 2])
                nreg = nregs[t % 4]
                nc.gpsimd.reg_load(nreg, cntt[0:1, t:t + 1])
                _my_dma_gather(
                    nc.gpsimd,
                    g[:].rearrange("p (k w) -> p k w", w=ROWW),
                    tab[:, :],
                    cmb[:, 0:kt * 8],
                    kt * P, nreg, ROWW, ROWW, t % 4)
                return {"g": g, "cmb": cmb, "kt": kt}

            def s2_softmax(st):
                """e = leaky(a_s + a_d) + mask; w = exp(e) (no max-sub)."""
                kt = st["kt"]
                g, cmb = st["g"], st["cmb"]
                msk_t = cmb[:, kt * 8:kt * 16].bitcast(FP)       # [P, kt*4]
                adcol = cmb[:, kt * 16:kt * 16 + 2].bitcast(FP)  # [P, 1]
                g32 = g[:].bitcast(FP)
                as_view = g32.rearrange("p (k u) -> p k u", u=64)[:, :, 32:36]
                lrl = scp.tile([P, kt * 4], FP, tag="lrl", name="lrl")
                nc.scalar.activation(
                    lrl[:].rearrange("p (k u) -> p k u", u=4), as_view,
                    mybir.ActivationFunctionType.Prelu,
                    bias=adcol, scale=1.0, alpha=NEG_SLOPE)
                e = scp.tile([P, kt * 4], FP, tag="e", name="e")
                nc.vector.tensor_tensor(out=e[:], in0=lrl[:], in1=msk_t,
                                        op=mybir.AluOpType.add)
                wts = scp.tile([P, kt * 4], FP, tag="w", name="wts")
                den = scp.tile([P, 1], FP, tag="den", name="den")
                nc.scalar.activation(
                    wts[:], e[:], mybir.ActivationFunctionType.Exp,
                    bias=0.0, scale=1.0, accum_out=den[:])
                st["wts"], st["den"] = wts, den

            def s3_aggregate(st, t):
                """num = sum w*h; normalize; layer head (DVE + PE)."""
                kt = st["kt"]
                g, wts, den = st["g"], st["wts"], st["den"]
                inv = scp.tile([P, 1], FP, tag="inv", name="inv")
                nc.vector.reciprocal(inv[:], den[:])
                h_view = g[:].rearrange("p (k u) -> p k u",
                                        u=ROWW)[:, :, 0:4 * HID]
                prod = oup.tile([P, kt * 4 * HID], BF, tag="prod", name="prod")
                nc.vector.tensor_tensor(
                    out=prod[:].rearrange("p (k s w) -> p k s w", s=4, w=HID),
                    in0=h_view.rearrange("p k (s w) -> p k s w", w=HID),
                    in1=wts[:].rearrange("p (k s) -> p k s", s=4)
                        .to_broadcast([P, kt, 4, HID]),
                    op=mybir.AluOpType.mult)
                num = oup.tile([P, HID], FP, tag="num", name="num")
                pv = prod[:].rearrange("p (k w) -> p w k", w=HID)
                nc.vector.tensor_reduce(num[:], pv, axis=mybir.AxisListType.X,
                                        op=mybir.AluOpType.add)
                if layer == 1:
                    ht = oup.tile([P, HID + 2], FP, tag="ht", name="ht")
                    # h' = relu(num/den + b1)
                    nc.vector.scalar_tensor_tensor(
                        out=ht[:, 0:HID], in0=num[:], scalar=inv[:],
                        in1=vt[:, 0:HID], op0=mybir.AluOpType.mult,
                        op1=mybir.AluOpType.add)
                    nc.vector.tensor_scalar_max(ht[:, 0:HID], ht[:, 0:HID], 0.0)
                    pT = ps.tile([HID, P], FP, space="PSUM", tag="pT",
                                 name="pT")
                    nc.tensor.transpose(pT[:], ht[:, 0:HID], ident[:])
                    hT = oup.tile([HID, P], FP, tag="hT", name="hT")
                    nc.scalar.copy(hT[:], pT[:])
                    pu = ps.tile([P, 2], FP, space="PSUM", tag="pu", name="pu")
                    nc.tensor.matmul(pu[:], lhsT=hT[:], rhs=uvt[:],
                                     start=True, stop=True)
                    st["ht"], st["pu"] = ht, pu
                else:
                    pT = ps.tile([HID, P], FP, space="PSUM", tag="pT",
                                 name="pT")
                    nc.tensor.transpose(pT[:], num[:], ident[:])
                    nT = oup.tile([HID, P], FP, tag="nT", name="nT")
                    nc.scalar.copy(nT[:], pT[:])
                    p2 = ps.tile([P, C_OUT], FP, space="PSUM", tag="p2",
                                 name="p2")
                    nc.tensor.matmul(p2[:], lhsT=nT[:], rhs=w2t[:],
                                     start=True, stop=True)
                    st["p2"], st["inv"] = p2, inv

            def s4_writeback(st, t, obufs):
                """L1: pack [h'|a_s2|a_d2] and DMA out. L2: z into group buf;
                batched log_softmax + DMA once per GS tiles."""
                if layer == 1:
                    ht, pu = st["ht"], st["pu"]
                    nc.scalar.copy(ht[:, HID:HID + 2], pu[:])
                    nc.sync.dma_start(out=hasd2[t * P:(t + 1) * P, :],
                                      in_=ht[:])
                    return
                gidx = t % GS
                if gidx == 0:
                    obufs[0] = oup.tile([P, GS * C_OUT], FP, tag="obuf",
                                        name="obuf", bufs=2)
                ob = obufs[0]
                # z = (num @ W2)/den + b2
                nc.vector.scalar_tensor_tensor(
                    out=ob[:, gidx * C_OUT:(gidx + 1) * C_OUT], in0=st["p2"][:],
                    scalar=st["inv"][:], in1=vt[:, 0:C_OUT],
                    op0=mybir.AluOpType.mult, op1=mybir.AluOpType.add)
                if gidx != GS - 1:
                    return
                # batched log_softmax over [P, GS, C_OUT] (z bounded, no
                # max-sub needed before exp)
                t0 = t - GS + 1
                ex = oup.tile([P, GS * C_OUT], FP, tag="ex", name="ex")
                nc.scalar.activation(ex[:], ob[:],
                                     mybir.ActivationFunctionType.Exp)
                se = scp.tile([P, GS], FP, tag="se", name="se")
                nc.vector.tensor_reduce(
                    se[:], ex[:].rearrange("p (g c) -> p g c", c=C_OUT),
                    axis=mybir.AxisListType.X, op=mybir.AluOpType.add)
                ls = scp.tile([P, GS], FP, tag="ls", name="ls")
                nc.scalar.activation(ls[:], se[:],
                                     mybir.ActivationFunctionType.Ln)
                nc.vector.tensor_tensor(
                    out=ob[:].rearrange("p (g c) -> p g c", c=C_OUT),
                    in0=ob[:].rearrange("p (g c) -> p g c", c=C_OUT),
                    in1=ls[:].to_broadcast([P, GS, C_OUT]),
                    op=mybir.AluOpType.subtract)
                nc.sync.dma_start(
                    out=y[t0 * P:(t0 + GS) * P, :]
                        .rearrange("(g p) c -> p g c", p=P),
                    in_=ob[:].rearrange("p (g c) -> p g c", c=C_OUT))

            def body(it=None):
                DEPTH = 3
                stages = {}
                obufs = [None]
                for i in range(T_TILES + DEPTH):
                    if i < T_TILES:
                        stages[i] = s1_gather(i)
                    if bench_mode == 1:
                        continue
                    if DEPTH >= 1 and 0 <= i - 1 < T_TILES:
                        s2_softmax(stages[i - 1])
                    if bench_mode == 2:
                        continue
                    if 0 <= i - 2 < T_TILES:
                        s3_aggregate(stages[i - 2], i - 2)
                    if 0 <= i - 3 < T_TILES:
                        s4_writeback(stages[i - 3], i - 3, obufs)
                        del stages[i - 3]

            if repeat > 1:
                with tc.For_i(0, repeat):
                    body()
            else:
                body()
    nc.compile()
    return nc


# ---------------------------------------------------------------------------
# Main entry
# ---------------------------------------------------------------------------

LAST_TIMINGS = {}
LAST_STATS = {}


def _run_retry(nc, in_maps, cores):
    try:
        return run_bass_kernel_spmd(nc, in_maps, cores)
    except Exception:
        # transient accelerator-unrecoverable states heal on retry
        return run_bass_kernel_spmd(nc, in_maps, cores)


def kernel(x, edge_index, W1, att_src1, att_dst1, b1, W2, att_src2, att_dst2, b2):
    import time as _time
    x = np.asarray(x, np.float32)
    W1 = np.asarray(W1, np.float32)
    W2 = np.asarray(W2, np.float32)
    att_src1 = np.asarray(att_src1, np.float32)
    att_dst1 = np.asarray(att_dst1, np.float32)
    att_src2 = np.asarray(att_src2, np.float32)
    att_dst2 = np.asarray(att_dst2, np.float32)
    b1 = np.asarray(b1, np.float32)
    b2 = np.asarray(b2, np.float32)

    import jax.numpy as jnp

    def to_bf16(a):
        return np.asarray(jnp.asarray(a, dtype=jnp.bfloat16))

    def bf16_bits(a):
        return np.asarray(jnp.asarray(a, dtype=jnp.bfloat16)).view(np.uint16)

    print("preprocess...", flush=True)
    _t = _time.time()
    K, coff, cores_data = _build_schedule(edge_index)
    LAST_STATS["descs_per_core"] = [int(cd["counts"].sum())
                                    for cd in cores_data]
    LAST_STATS["sumK"] = int(K.sum())
    LAST_TIMINGS["preprocess"] = _time.time() - _t

    # ---- launch 1: transform -------------------------------------------
    print("build1...", flush=True)
    nc1 = _build_transform()
    xT_bf = to_bf16(np.ascontiguousarray(x.T))
    w1r = np.ascontiguousarray(to_bf16(W1).reshape(F_IN // P, P, HID))
    attuv = np.stack([att_src1, att_dst1], axis=1).astype(np.float32)
    in1 = [{"xt": np.ascontiguousarray(xT_bf[:, c * SH:(c + 1) * SH]),
            "w1": w1r, "attuv": attuv}
           for c in range(NCORES)]
    _t = _time.time()
    r1 = _run_retry(nc1, in1, list(range(NCORES)))
    LAST_TIMINGS["launch1"] = _time.time() - _t
    print("launch1 done", flush=True)
    hasd1 = np.concatenate(
        [r1.results[c]["hasd"][:SH] for c in range(NCORES)], axis=0)  # [N,18]
    tab1 = _pack_table(bf16_bits(hasd1[:, 0:HID]), hasd1[:, HID])

    # ---- launch 2: layer-1 aggregation ---------------------------------
    print("build2...", flush=True)
    nc2 = _build_aggregate(K, coff, layer=1)
    u2 = W2 @ att_src2
    v2 = W2 @ att_dst2
    vecs1 = np.tile(b1[None, :], (P, 1)).astype(np.float32)
    uv2 = np.stack([u2, v2], axis=1).astype(np.float32)
    combs1 = _fill_ad(cores_data, K, coff, hasd1[:, HID + 1].copy())
    in2 = [{"tab": tab1, "comb": combs1[c], "cnts": cores_data[c]["counts"],
            "vecs": vecs1, "uv": uv2} for c in range(NCORES)]
    _t = _time.time()
    r2 = _run_retry(nc2, in2, list(range(NCORES)))
    LAST_TIMINGS["launch2"] = _time.time() - _t
    print("launch2 done", flush=True)
    hasd2 = np.empty((N, HID + 2), np.float32)
    for c in range(NCORES):
        ids = c * SH + cores_data[c]["order"]
        hasd2[ids] = r2.results[c]["hasd2"][:SH]
    tab2 = _pack_table(bf16_bits(hasd2[:, 0:HID]), hasd2[:, HID])

    # ---- launch 3: layer-2 aggregation + classifier --------------------
    print("build3...", flush=True)
    nc3 = _build_aggregate(K, coff, layer=2)
    vecs2 = np.tile(b2[None, :], (P, 1)).astype(np.float32)
    combs2 = _fill_ad(cores_data, K, coff, hasd2[:, HID + 1].copy())
    in3 = [{"tab": tab2, "comb": combs2[c], "cnts": cores_data[c]["counts"],
            "vecs": vecs2, "w2": W2} for c in range(NCORES)]
    _t = _time.time()
    r3 = _run_retry(nc3, in3, list(range(NCORES)))
    LAST_TIMINGS["launch3"] = _time.time() - _t
    print("launch3 done", flush=True)

    out = np.zeros((N, C_OUT), np.float32)
    for c in range(NCORES):
        out[c * SH + cores_data[c]["order"]] = r3.results[c]["y"][:SH, :]
    return out
